# revision 1
# baseline (speedup 1.0000x reference)
"""TRN2 Bass kernel for nn_LocalPoolPointnetPPFusion (batch-parallel, 8 cores).

Per-core pipeline, feature-major activations [128, 8192] bf16, biases deferred.
The two streams (geometry g / articulation c) are INTERLEAVED at token level
for all pooling DMA: each token row in the point-major (PM) buffers is 512B
(g-features then c-features), so every gather descriptor serves both streams
at once -- descriptor generation on the Q7 is the serial bottleneck
(~7.5ns/descriptor), so halving descriptor count halves pool time.

  net0' = p @ wp (+ p2 @ wp2 for corr stream)          (biases deferred)
  5 resblocks per stream; between blocks:
    net_g/net_c --xbar--> npm2 [128, 65, 2, 128] (rank 64 = zeros = ZROW)
    per plane: SBUF-source transpose-gathers (elem 256 = both streams) build
    occupancy-sorted strips, prefix TT-max -> per-bin max (FM, both streams)
    --xbar--> tbl2 PM -> one expand gather -> pooled2 [128, 2, T]; 3 planes
    summed.
  final stage: same strips with fp32 prefix add (per-bin sums), cast bf16,
    @ fc_w per stream on PE -> per-bin [bins, C] sums -> plain DMA to compact
    HBM tensors [128, G, 2, C] (no scatter; host scatters into R*R grids).
  host folds deferred biases + fc bias + 1/cnt + transposes to [C, R, R].

Timing mode (measure_hw_time): the rep loop is a device-side tc.For_i hardware
loop, so the NEFF stays the same size for any rep count and wall-clock
differencing isolates true per-iteration HW time.
"""
import sys
sys.path.insert(0, "/opt/trn_rl_repo")

import numpy as np
import ml_dtypes

BF = ml_dtypes.bfloat16
F32 = np.float32

B, T, H, C, R = 8, 8192, 128, 128, 128
NB = 5
NPLANES = 3
PLANE_COLS = ((0, 2), (0, 1), (1, 2))
ZROW = T          # zero-token index (rank 64 of npm2)


def compute_idx_lists(p_np):
    import jax
    import jax.numpy as jnp
    cpu = jax.devices("cpu")[0]
    out = []
    with jax.default_device(cpu):
        pj = jnp.asarray(p_np)
        for cols in PLANE_COLS:
            xy = pj[..., jnp.array(cols)] / (1.0 + 0.0 + 1e-3) + 0.5
            xy = jnp.clip(xy, 0.0, 1.0 - 1e-3)
            g = jnp.floor(xy * R).astype(jnp.int32)
            out.append(np.asarray(g[..., 0] + R * g[..., 1]))
    return out


def wrap_idxs(flat):
    """token i -> idxs[i%16, i//16]; replicated to 128 partitions."""
    flat = np.asarray(flat, np.int64)
    n = len(flat)
    assert n % 16 == 0
    a = flat.reshape(n // 16, 16).T.astype(np.int16)
    return np.tile(np.ascontiguousarray(a), (8, 1))


def ceil128(x):
    return max((int(x) + 127) // 128 * 128, 128)


class PlanePrep:
    def __init__(self, idx):
        self.idx = idx
        cnt = np.bincount(idx, minlength=R * R)
        self.cnt = cnt
        occ = np.where(cnt > 0)[0]
        order = np.argsort(-cnt[occ], kind="stable")
        self.bins_sorted = occ[order]
        self.n_occ = len(occ)
        self.occ_sorted = cnt[self.bins_sorted]
        sort_by_bin = np.argsort(idx, kind="stable")
        starts = np.searchsorted(idx[sort_by_bin], self.bins_sorted)
        self.members = [sort_by_bin[s:s + k] for s, k in zip(starts, self.occ_sorted)]
        slot_of_bin = np.full(R * R, -1, np.int64)
        slot_of_bin[self.bins_sorted] = np.arange(self.n_occ)
        self.pidx = slot_of_bin[idx]
        self.R_max = int(self.occ_sorted[0])
        self.n_r = [int((self.occ_sorted >= r).sum()) for r in range(1, self.R_max + 1)]

    def nr(self, r):
        return self.n_r[r - 1] if r <= self.R_max else 0

    def round_ids(self, r, width, sum_pad):
        ids = np.full(width, ZROW, np.int64)
        nr = self.nr(r)
        for s in range(min(nr, width)):
            ids[s] = self.members[s][r - 1]
        if not sum_pad:
            for s in range(nr, width):
                ids[s] = self.members[s][0] if s < self.n_occ else ZROW
        return ids


def _build(inputs, preps, REPS=1, timing=False):
    """Build program + per-core in_maps. timing=True uses internal outputs
    and wraps the rep body in a device-side For_i loop."""
    import concourse.bacc as bacc
    import concourse.tile as tile
    from concourse import mybir

    p = np.asarray(inputs["p"], F32)
    p2 = np.asarray(inputs["p2"], F32)

    N1P = [max(ceil128(preps[b][pl].n_occ) for b in range(B)) for pl in range(NPLANES)]
    RMAX = [max(preps[b][pl].R_max for b in range(B)) for pl in range(NPLANES)]
    CR = []
    for pl in range(NPLANES):
        CR.append([ceil128(max(preps[b][pl].nr(r) for b in range(B)))
                   for r in range(2, RMAX[pl] + 1)])
    MAXCR = max(max(c) if c else 128 for c in CR)
    MAXN1P = max(N1P)
    CHUNK = 1024 if MAXN1P <= 2048 else 512  # mean-stage bin chunk
    PCHUNK = 2048 if MAXN1P <= 2048 else 1024  # pool strip bin chunk
    SRW = max(MAXCR, CHUNK)       # strip tile width
    EXCH = 1024                   # expand gather token chunk
    # deeper staging buffers when the bin tables are small enough to leave
    # SBUF headroom: keeps the Pool engine generating descriptors instead of
    # waiting on DVE consumers of the previous chunk
    small_stats = MAXN1P <= 2048
    GPB = 2 if small_stats else 1
    SRB = 3 if small_stats else 2

    def stream_host(pref, base_bias):
        w0 = np.asarray(inputs[f"{pref}_w0"], F32)
        b0 = np.asarray(inputs[f"{pref}_b0"], F32)
        w1 = np.asarray(inputs[f"{pref}_w1"], F32)
        b1 = np.asarray(inputs[f"{pref}_b1"], F32)
        ws = np.asarray(inputs[f"{pref}_ws"], F32)
        relu_bias = []
        Bp = base_bias
        for i in range(NB):
            if i == 0:
                bias_in = Bp
                relu_bias.append((bias_in[:H].copy(), bias_in[H:].copy()))
            else:
                bias_in = np.concatenate([Bp, 3.0 * Bp])
                relu_bias.append((Bp.copy(), 3.0 * Bp))
            Bp = b1[i] + bias_in @ ws[i]
        return dict(w0=w0, b0=b0, w1=w1, ws=ws, relu_bias=relu_bias, B_final=Bp)

    wp = np.asarray(inputs["wp"], F32)
    bp = np.asarray(inputs["bp"], F32)
    wp2 = np.asarray(inputs["wp2"], F32)
    bp2 = np.asarray(inputs["bp2"], F32)
    sh_host = {"g": stream_host("blk", bp.copy()), "c": stream_host("blkc", bp + bp2)}
    fc_w = {"g": np.asarray(inputs["fc_c_w"], F32),
            "c": np.asarray(inputs["fc_cc_w"], F32)}
    fc_b = {"g": np.asarray(inputs["fc_c_b"], F32),
            "c": np.asarray(inputs["fc_cc_b"], F32)}
    cvec = {s: sh_host[s]["B_final"] @ fc_w[s] + fc_b[s] for s in ("g", "c")}

    nc = bacc.Bacc("TRN2", target_bir_lowering=False, debug=False, num_devices=B)
    dt = mybir.dt

    def din(name, shape, dtype):
        return nc.dram_tensor(name, shape, dtype, kind="ExternalInput")

    pT_d = din("pT", [3, T], dt.bfloat16)
    p2T_d = din("p2T", [3, T], dt.bfloat16)
    wp_d = din("wp", [3, 2 * H], dt.bfloat16)
    wp2_d = din("wp2", [3, 2 * H], dt.bfloat16)
    wpk_d = {}
    for s in ("g", "c"):
        wpk_d[s] = dict(
            w0=din(f"{s}_w0", [H, NB, 2 * H], dt.bfloat16),
            w1=din(f"{s}_w1", [H, NB, H], dt.bfloat16),
            ws=din(f"{s}_ws", [H, NB, 2 * H], dt.bfloat16),
            rb=din(f"{s}_rb", [H, NB, 2], dt.float32),
            b0=din(f"{s}_b0", [H, NB], dt.float32),
            fcw=din(f"{s}_fcw", [H, C], dt.bfloat16),
        )
    g1_d = [din(f"g1_{pl}", [128, N1P[pl] // 16], dt.int16) for pl in range(NPLANES)]
    gmax_d = [[din(f"gmax_{pl}_{r}", [128, CR[pl][r - 2] // 16], dt.int16)
               for r in range(2, RMAX[pl] + 1)] for pl in range(NPLANES)]
    gsum_d = [[din(f"gsum_{pl}_{r}", [128, CR[pl][r - 2] // 16], dt.int16)
               for r in range(2, RMAX[pl] + 1)] for pl in range(NPLANES)]
    pidx_d = [din(f"pidx_{pl}", [128, T // 16], dt.int16) for pl in range(NPLANES)]

    out_kind = "Internal" if timing else "ExternalOutput"
    # compact per-bin sums, both streams interleaved: [p, g, s, C] = rank g*128+p
    sums_d = [nc.dram_tensor(f"sums_{pl}", [128, N1P[pl] // 128, 2, C], dt.float32,
                             kind=out_kind) for pl in range(NPLANES)]
    chk_d = nc.dram_tensor("chk", [128, 128], dt.bfloat16, kind="ExternalOutput") \
        if timing else None

    with tile.TileContext(nc) as tc:
        with tc.tile_pool(name="const", bufs=1) as constp, \
             tc.tile_pool(name="act", bufs=1) as actp, \
             tc.tile_pool(name="pooledp", bufs=1) as pooledp, \
             tc.tile_pool(name="small", bufs=2) as smallp, \
             tc.tile_pool(name="sr", bufs=SRB) as srp, \
             tc.tile_pool(name="gp", bufs=GPB) as gp, \
             tc.tile_pool(name="npm", bufs=1) as npmp, \
             tc.tile_pool(name="pm", bufs=1) as pmp, \
             tc.tile_pool(name="meanp", bufs=1) as meanp, \
             tc.tile_pool(name="psum", bufs=2, space="PSUM") as psump:

            wp_t = constp.tile([3, 2 * H], dt.bfloat16)
            wp2_t = constp.tile([3, 2 * H], dt.bfloat16)
            nc.sync.dma_start(wp_t[:], wp_d[:])
            nc.sync.dma_start(wp2_t[:], wp2_d[:])
            W = {}
            for s in ("g", "c"):
                W[s] = dict(
                    w0=constp.tile([H, NB, 2 * H], dt.bfloat16, tag=f"{s}w0", name=f"{s}w0"),
                    w1=constp.tile([H, NB, H], dt.bfloat16, tag=f"{s}w1", name=f"{s}w1"),
                    ws=constp.tile([H, NB, 2 * H], dt.bfloat16, tag=f"{s}ws", name=f"{s}ws"),
                    rb=constp.tile([H, NB, 2], dt.float32, tag=f"{s}rb", name=f"{s}rb"),
                    b0=constp.tile([H, NB], dt.float32, tag=f"{s}b0", name=f"{s}b0"),
                    fcw=constp.tile([H, C], dt.bfloat16, tag=f"{s}fcw", name=f"{s}fcw"),
                )
                for k, t in W[s].items():
                    nc.sync.dma_start(t[:], wpk_d[s][k][:])
            g1_t, gmax_t, gsum_t, pidx_t = [], [], [], []
            for pl in range(NPLANES):
                g1_t.append(constp.tile([128, N1P[pl] // 16], dt.int16,
                                        tag=f"g1{pl}", name=f"g1t{pl}"))
                pidx_t.append(constp.tile([128, T // 16], dt.int16,
                                          tag=f"pi{pl}", name=f"pit{pl}"))
                nc.sync.dma_start(g1_t[pl][:], g1_d[pl][:])
                nc.sync.dma_start(pidx_t[pl][:], pidx_d[pl][:])
                gm, gs = [], []
                for j in range(RMAX[pl] - 1):
                    tm = constp.tile([128, CR[pl][j] // 16], dt.int16,
                                     tag=f"gm{pl}_{j}", name=f"gmt{pl}_{j}")
                    ts_ = constp.tile([128, CR[pl][j] // 16], dt.int16,
                                      tag=f"gs{pl}_{j}", name=f"gst{pl}_{j}")
                    nc.sync.dma_start(tm[:], gmax_d[pl][j][:])
                    nc.sync.dma_start(ts_[:], gsum_d[pl][j][:])
                    gm.append(tm)
                    gs.append(ts_)
                gmax_t.append(gm)
                gsum_t.append(gs)

            def sbuf_gather2(dst_ap, src_pm, idxs_ap, n):
                """Interleaved SBUF-source transpose gather: token i's 512B row
                (g then c features) at [i%128, i//128, :, :]."""
                nc.gpsimd.dma_gather(
                    dst_ap, src_pm, idxs_ap, n, n, 2 * H,
                    transpose=True, single_packet=False,
                    sbuf_tokens_per_rank=128,
                    sbuf_free_dim_per_rank=2 * H * 2,
                )

            def sgview(flat_tile, w):
                """[128, 2w] flat slice viewed as [128, 2, w] (contiguous)."""
                return flat_tile[:, :2 * w].rearrange("p (s w) -> p s w", s=2)

            def make_net_pm2(net_g, net_c):
                """Transpose both streams into interleaved PM [128, 65, 2, 128];
                rank 64 = zeros (ZROW)."""
                npm2 = npmp.tile([128, 65, 2, H], dt.bfloat16, tag="npm", name="npm")
                nc.vector.memset(npm2[:, 64, :, :], 0.0)
                nc.sync.dma_start_transpose(npm2[:, :64, 0, :], net_g[:])
                nc.sync.dma_start_transpose(npm2[:, :64, 1, :], net_c[:])
                return npm2

            def pool_local2(npm2):
                """All 3 planes, both streams at once -> pooled2 [128, 2, T]."""
                pooled2 = pooledp.tile([128, 2, T], dt.bfloat16,
                                       tag="pooled2", name="pooled2")
                for pl in range(NPLANES):
                    n1 = N1P[pl]
                    tbl2 = pmp.tile([128, MAXN1P // 128, 2, H], dt.bfloat16,
                                    tag="tbl2", name="tbl2")
                    for c0 in range(0, n1, PCHUNK):
                        wch = min(PCHUNK, n1 - c0)
                        s12f = pmp.tile([128, 2 * PCHUNK], dt.bfloat16,
                                        tag="s12", name="s12")
                        s12 = sgview(s12f, wch)
                        sbuf_gather2(s12, npm2[:],
                                     g1_t[pl][:, c0 // 16:(c0 + wch) // 16], wch)
                        for j in range(RMAX[pl] - 1):
                            w = min(CR[pl][j], c0 + wch) - c0
                            if w <= 0:
                                continue
                            sr2f = srp.tile([128, 2 * SRW], dt.bfloat16,
                                            tag="sr", name="sr")
                            sr2 = sgview(sr2f, w)
                            sbuf_gather2(sr2, npm2[:],
                                         gmax_t[pl][j][:, c0 // 16:(c0 + w) // 16], w)
                            nc.vector.tensor_tensor(
                                out=s12[:, :, :w], in0=s12[:, :, :w],
                                in1=sr2[:], op=mybir.AluOpType.max)
                        nc.sync.dma_start_transpose(
                            tbl2[:, c0 // 128:(c0 + wch) // 128, 0, :],
                            s12f[:, :wch])
                        nc.sync.dma_start_transpose(
                            tbl2[:, c0 // 128:(c0 + wch) // 128, 1, :],
                            s12f[:, wch:2 * wch])
                    # chunked expand: keeps per-instruction SWDGE descriptor
                    # footprint inside the ring carveout
                    for t0 in range(0, T, EXCH):
                        g2 = gp.tile([128, 2, EXCH], dt.bfloat16,
                                     tag="g2", name="g2")
                        sbuf_gather2(g2[:], tbl2[:],
                                     pidx_t[pl][:, t0 // 16:(t0 + EXCH) // 16],
                                     EXCH)
                        if pl == 0:
                            nc.vector.tensor_copy(pooled2[:, :, t0:t0 + EXCH],
                                                  g2[:])
                        else:
                            nc.vector.tensor_tensor(
                                out=pooled2[:, :, t0:t0 + EXCH],
                                in0=pooled2[:, :, t0:t0 + EXCH],
                                in1=g2[:], op=mybir.AluOpType.add)
                return pooled2

            def resblock(s, i, xa, xb):
                """In-place: writes output into xa. Returns xa."""
                w = W[s]
                ba_ap = w["rb"][:, i, 0:1]
                bb_ap = w["rb"][:, i, 1:2]
                for nt in range(T // 512):
                    sl = slice(nt * 512, (nt + 1) * 512)
                    ra = smallp.tile([H, 512], dt.bfloat16, tag="ra", name="ra")
                    rb_ = smallp.tile([H, 512], dt.bfloat16, tag="rb", name="rb")
                    nc.vector.tensor_scalar(out=ra[:], in0=xa[:, sl], scalar1=ba_ap,
                                            scalar2=0.0, op0=mybir.AluOpType.add,
                                            op1=mybir.AluOpType.max)
                    nc.vector.tensor_scalar(out=rb_[:], in0=xb[:, sl], scalar1=bb_ap,
                                            scalar2=0.0, op0=mybir.AluOpType.add,
                                            op1=mybir.AluOpType.max)
                    ph = psump.tile([H, 512], dt.float32, tag="ph", name="ph")
                    nc.tensor.matmul(ph[:], w["w0"][:, i, :H], ra[:],
                                     start=True, stop=False)
                    nc.tensor.matmul(ph[:], w["w0"][:, i, H:], rb_[:],
                                     start=False, stop=True)
                    h = smallp.tile([H, 512], dt.bfloat16, tag="h", name="h")
                    nc.scalar.activation(h[:], ph[:], mybir.ActivationFunctionType.Relu,
                                         bias=w["b0"][:, i:i + 1], scale=1.0)
                    po = psump.tile([H, 512], dt.float32, tag="po", name="po")
                    nc.tensor.matmul(po[:], w["w1"][:, i, :], h[:],
                                     start=True, stop=False)
                    nc.tensor.matmul(po[:], w["ws"][:, i, :H], xa[:, sl],
                                     start=False, stop=False)
                    nc.tensor.matmul(po[:], w["ws"][:, i, H:], xb[:, sl],
                                     start=False, stop=True)
                    nc.scalar.activation(xa[:, sl], po[:],
                                         mybir.ActivationFunctionType.Copy)
                return xa

            def mean_stage2(npm2):
                """Per-bin sums of both streams -> fc_w matmuls -> compact HBM."""
                for pl in range(NPLANES):
                    n1 = N1P[pl]
                    for c0 in range(0, n1, CHUNK):
                        wch = min(CHUNK, n1 - c0)
                        s1ff = srp.tile([128, 2 * SRW], dt.bfloat16,
                                        tag="sr", name="s1f")
                        s1f = sgview(s1ff, wch)
                        sbuf_gather2(s1f, npm2[:],
                                     g1_t[pl][:, c0 // 16:(c0 + wch) // 16], wch)
                        acc = meanp.tile([128, 2, CHUNK], dt.float32,
                                         tag="acc", name="acc")
                        nc.vector.tensor_copy(acc[:, :, :wch], s1f[:])
                        for j in range(RMAX[pl] - 1):
                            w = min(CR[pl][j], c0 + wch) - c0
                            if w <= 0:
                                continue
                            srff = srp.tile([128, 2 * SRW], dt.bfloat16,
                                            tag="sr", name="srf")
                            srf = sgview(srff, w)
                            sbuf_gather2(srf, npm2[:],
                                         gsum_t[pl][j][:, c0 // 16:(c0 + w) // 16], w)
                            nc.vector.tensor_tensor(out=acc[:, :, :w],
                                                    in0=acc[:, :, :w],
                                                    in1=srf[:],
                                                    op=mybir.AluOpType.add)
                        accbf = srp.tile([128, 2 * SRW], dt.bfloat16,
                                         tag="sr", name="accb")
                        accb = sgview(accbf, wch)
                        nc.vector.tensor_copy(accb[:], acc[:, :, :wch])
                        for ch2 in range((wch // 128 + 1) // 2):
                            nch = min(2, wch // 128 - ch2 * 2)
                            sums = meanp.tile([128, 2, 2, C], dt.float32,
                                              tag="sums", name="sums")
                            for si, s in enumerate(("g", "c")):
                                pb = psump.tile([128, 512], dt.float32,
                                                tag="ph", name="pb")
                                for k in range(nch):
                                    chunk = ch2 * 2 + k
                                    nc.tensor.matmul(
                                        pb[:, k * C:(k + 1) * C],
                                        accb[:, si, chunk * 128:(chunk + 1) * 128],
                                        W[s]["fcw"][:], start=True, stop=True)
                                nc.vector.tensor_copy(
                                    sums[:, :nch, si, :],
                                    pb[:, :nch * C].rearrange(
                                        "p (a f) -> p a f", a=nch))
                            nc.sync.dma_start(
                                sums_d[pl][:, c0 // 128 + ch2 * 2:
                                           c0 // 128 + ch2 * 2 + nch, :, :],
                                sums[:, :nch, :, :])

            # ---------------- schedule ----------------
            net = {}

            def one_rep():
                pT_t = npmp.tile([3, T], dt.bfloat16, tag="pT", name="pT_t")
                p2T_t = npmp.tile([3, T], dt.bfloat16, tag="p2T", name="p2T_t")
                nc.sync.dma_start(pT_t[:], pT_d[:])
                nc.sync.dma_start(p2T_t[:], p2T_d[:])
                x0b2 = pooledp.tile([128, 2, T], dt.bfloat16,
                                    tag="pooled2", name="x0b2")
                xa = {"g": actp.tile([H, T], dt.bfloat16, tag="netg", name="x0g0"),
                      "c": actp.tile([H, T], dt.bfloat16, tag="netc", name="x0c0")}
                for m in range(2):
                    for nt in range(T // 512):
                        sl = slice(nt * 512, (nt + 1) * 512)
                        ps_g = psump.tile([H, 512], dt.float32, tag="ph", name="ps_g")
                        ps_c = psump.tile([H, 512], dt.float32, tag="po", name="ps_c")
                        nc.tensor.matmul(ps_g[:], wp_t[:, m * H:(m + 1) * H],
                                         pT_t[:, sl], start=True, stop=True)
                        nc.tensor.matmul(ps_c[:], wp2_t[:, m * H:(m + 1) * H],
                                         p2T_t[:, sl], start=True, stop=True)
                        dst_g = xa["g"][:, sl] if m == 0 else x0b2[:, 0, sl]
                        dst_c = xa["c"][:, sl] if m == 0 else x0b2[:, 1, sl]
                        nc.scalar.activation(dst_g, ps_g[:],
                                             mybir.ActivationFunctionType.Copy)
                        nc.vector.tensor_tensor(out=dst_c, in0=dst_g,
                                                in1=ps_c[:], op=mybir.AluOpType.add)

                for si, s in enumerate(("g", "c")):
                    net[s] = resblock(s, 0, xa[s], x0b2[:, si, :])
                for i in range(1, NB):
                    npm2 = make_net_pm2(net["g"], net["c"])
                    pooled2 = pool_local2(npm2)
                    for si, s in enumerate(("g", "c")):
                        net[s] = resblock(s, i, net[s], pooled2[:, si, :])
                npm_f = make_net_pm2(net["g"], net["c"])
                mean_stage2(npm_f)

            if timing:
                # device-side repetition: NEFF size stays constant across
                # REPS so wall-differencing isolates per-iteration HW time
                with tc.For_i(0, REPS):
                    one_rep()
            else:
                one_rep()

            if timing:
                chk_t = constp.tile([128, 128], dt.bfloat16)
                nc.vector.tensor_copy(chk_t[:], net["g"][:, :128])
                nc.sync.dma_start(chk_d[:], chk_t[:])

    nc.compile()

    in_maps = []
    for b in range(B):
        im = {
            "pT": np.ascontiguousarray(p[b].T).astype(BF),
            "p2T": np.ascontiguousarray(p2[b].T).astype(BF),
            "wp": wp.astype(BF), "wp2": wp2.astype(BF),
        }
        for s in ("g", "c"):
            sh = sh_host[s]
            w0pk = np.concatenate([sh["w0"][:, :H].transpose(1, 0, 2),
                                   sh["w0"][:, H:].transpose(1, 0, 2)], axis=2)
            wspk = np.concatenate([sh["ws"][:, :H].transpose(1, 0, 2),
                                   sh["ws"][:, H:].transpose(1, 0, 2)], axis=2)
            w1pk = sh["w1"].transpose(1, 0, 2)
            rb = np.zeros((H, NB, 2), F32)
            for i, (ba, bb) in enumerate(sh["relu_bias"]):
                rb[:, i, 0] = ba
                rb[:, i, 1] = bb
            im[f"{s}_w0"] = np.ascontiguousarray(w0pk).astype(BF)
            im[f"{s}_w1"] = np.ascontiguousarray(w1pk).astype(BF)
            im[f"{s}_ws"] = np.ascontiguousarray(wspk).astype(BF)
            im[f"{s}_rb"] = rb
            im[f"{s}_b0"] = np.ascontiguousarray(sh["b0"].T).astype(F32)
            im[f"{s}_fcw"] = fc_w[s].astype(BF)
        for pl in range(NPLANES):
            pr = preps[b][pl]
            im[f"g1_{pl}"] = wrap_idxs(pr.round_ids(1, N1P[pl], sum_pad=True))
            for j, r in enumerate(range(2, RMAX[pl] + 1)):
                im[f"gmax_{pl}_{r}"] = wrap_idxs(pr.round_ids(r, CR[pl][j], sum_pad=False))
                im[f"gsum_{pl}_{r}"] = wrap_idxs(pr.round_ids(r, CR[pl][j], sum_pad=True))
            im[f"pidx_{pl}"] = wrap_idxs(pr.pidx)
        in_maps.append(im)

    return nc, in_maps, cvec


def _prep(inputs):
    p = np.asarray(inputs["p"], F32)
    idx_lists = compute_idx_lists(p)
    return [[PlanePrep(idx_lists[pl][b]) for pl in range(NPLANES)] for b in range(B)]


def kernel(**inputs):
    from concourse.bass_utils import run_bass_kernel_spmd

    preps = _prep(inputs)
    nc, in_maps, cvec = _build(inputs, preps, REPS=1, timing=False)
    res = run_bass_kernel_spmd(nc, in_maps, core_ids=list(range(B)))

    out = np.zeros((2 * NPLANES, B, C, R, R), F32)
    for b in range(B):
        for pl in range(NPLANES):
            pr = preps[b][pl]
            compact = np.asarray(res.results[b][f"sums_{pl}"], F32)  # [128,G,2,C]
            ranks = compact.transpose(1, 0, 2, 3).reshape(-1, 2, C)  # rank g*128+p
            cnt = pr.cnt.astype(F32)
            for si, s in enumerate(("g", "c")):
                grid = np.zeros((R * R, C), F32)
                grid[pr.bins_sorted] = ranks[:pr.n_occ, si]
                true_sums = grid + cnt[:, None] * cvec[s][None, :]
                mean = true_sums / np.clip(cnt, 1.0, None)[:, None]
                mean[cnt == 0] = 0.0
                out[si * NPLANES + pl, b] = mean.T.reshape(C, R, R)
    return out


def measure_hw_time(inputs, reps=1000, n_timing_runs=8):
    """Estimate per-iteration device time via in-kernel repetition differencing."""
    import time
    from concourse.bass_utils import run_bass_kernel_spmd

    preps = _prep(inputs)

    def runner(R_):
        nc, in_maps, _ = _build(inputs, preps, REPS=R_, timing=True)

        def once():
            t0 = time.perf_counter()
            run_bass_kernel_spmd(nc, in_maps, core_ids=list(range(B)))
            return time.perf_counter() - t0
        once()  # warm
        return min(once() for _ in range(n_timing_runs))

    t1 = runner(1)
    tR = runner(reps)
    per_iter = (tR - t1) / (reps - 1)
    return int(per_iter * 1e9), t1, tR


if __name__ == "__main__":
    import reference
    inputs = {k: np.asarray(v) for k, v in reference.setup_inputs().items()}
    result = kernel(**inputs)
    print("kernel output shape:", result.shape)



# revision 16
# speedup vs baseline: 1.5013x; 1.5013x over previous
"""TRN2 Bass kernel for nn_LocalPoolPointnetPPFusion (batch-parallel, 8 cores).

Sigma-reordered design: the global token storage order is plane-0's strip
order (member r of occupancy-sorted bin slot s, concatenated round-major).
Plane 0's pooling then needs NO gathers at all: per-bin max = prefix maxes
over contiguous FM slices of net, and the expand-back = contiguous slice
copies (DVE only). Only planes 1 and 2 use the SWDGE gather pipeline
(~8ns/idx on the Q7 Pool engine -- the serial bottleneck), roughly halving
Pool-engine time vs gathering all 3 planes.

SPMD constraint (one program, 8 cores): strip-round widths are padded to the
max profile across cores; hole positions duplicate the slot's member-0 token
(idempotent under max). For the final scatter-mean, duplicate contributions
are gathered separately (dup table, ZROW-padded) and emitted as a compact
`dups` output that the host subtracts from plane-0 bin sums.

The two streams (geometry g / articulation c) stay interleaved at token level
for all pooling DMA (512B per token row) so each descriptor serves both.
Expand gathers combine both planes per chunk (single instruction) via a
stacked bin table. Biases stay deferred exactly as in the previous design.

Timing mode (measure_hw_time): device-side tc.For_i rep loop + wall-clock
differencing isolates per-iteration HW time.
"""
import sys
sys.path.insert(0, "/opt/trn_rl_repo")

import numpy as np
import ml_dtypes

BF = ml_dtypes.bfloat16
F32 = np.float32

B, T, H, C, R = 8, 8192, 128, 128, 128
NB = 5
PLANE_COLS = ((0, 2), (0, 1), (1, 2))


def compute_idx_lists(p_np):
    import jax
    import jax.numpy as jnp
    cpu = jax.devices("cpu")[0]
    out = []
    with jax.default_device(cpu):
        pj = jnp.asarray(p_np)
        for cols in PLANE_COLS:
            xy = pj[..., jnp.array(cols)] / (1.0 + 0.0 + 1e-3) + 0.5
            xy = jnp.clip(xy, 0.0, 1.0 - 1e-3)
            g = jnp.floor(xy * R).astype(jnp.int32)
            out.append(np.asarray(g[..., 0] + R * g[..., 1]).astype(np.int64))
    return out


def wrap_idxs(flat):
    """idx i -> idxs[i%16, i//16]; replicated to 128 partitions."""
    flat = np.asarray(flat, np.int64)
    n = len(flat)
    assert n % 16 == 0
    assert flat.max() < 32768
    a = flat.reshape(n // 16, 16).T.astype(np.int16)
    return np.tile(np.ascontiguousarray(a), (8, 1))


def ceil128(x):
    return max((int(x) + 127) // 128 * 128, 128)


class Prep:
    """Bin structure over an index array restricted to `real` positions."""
    def __init__(self, idx_full, real_mask):
        pos = np.where(real_mask)[0]
        vals = idx_full[pos]
        cnt = np.bincount(vals, minlength=R * R)
        self.cnt = cnt
        occ = np.where(cnt > 0)[0]
        order = np.argsort(-cnt[occ], kind="stable")
        self.bins_sorted = occ[order]
        self.n_occ = len(occ)
        self.occ_sorted = cnt[self.bins_sorted]
        sort_by_bin = np.argsort(vals, kind="stable")
        starts = np.searchsorted(vals[sort_by_bin], self.bins_sorted)
        self.members = [pos[sort_by_bin[s:s + k]]
                        for s, k in zip(starts, self.occ_sorted)]
        slot_of_bin = np.full(R * R, -1, np.int64)
        slot_of_bin[self.bins_sorted] = np.arange(self.n_occ)
        self.pidx = slot_of_bin[idx_full]  # full length (holes inherit)
        self.R_max = int(self.occ_sorted[0])
        self.n_r = [int((self.occ_sorted >= r).sum())
                    for r in range(1, self.R_max + 1)]

    def nr(self, r):
        return self.n_r[r - 1] if r <= self.R_max else 0

    def round_ids(self, r, width, sum_pad, zrow):
        ids = np.full(width, zrow, np.int64)
        nr = self.nr(r)
        for s in range(min(nr, width)):
            ids[s] = self.members[s][r - 1]
        if not sum_pad:
            for s in range(nr, width):
                ids[s] = self.members[s][0] if s < self.n_occ else zrow
    # pads: sum_pad -> ZROW (gathers 0); else member[0] (idempotent for max)
        return ids


class Sigma:
    """Plane-0 strip-order permutation shared across 8 cores (max profile)."""
    def __init__(self, pr0_list):
        self.pr0 = pr0_list
        RMAX0 = max(pr.R_max for pr in pr0_list)
        self.RMAX0 = RMAX0
        self.nrmax = [max(pr.nr(r) for pr in pr0_list)
                      for r in range(1, RMAX0 + 1)]
        self.nrmin = [min(pr.nr(r) for pr in pr0_list)
                      for r in range(1, RMAX0 + 1)]
        noc_min = min(pr.n_occ for pr in pr0_list)
        assert all(self.nrmax[r - 1] <= noc_min for r in range(2, RMAX0 + 1))
        self.off = np.concatenate([[0], np.cumsum(self.nrmax)]).astype(np.int64)
        self.STRUCT_END = int(self.off[RMAX0])
        self.TP = (self.STRUCT_END + 511) // 512 * 512
        self.ZROW = self.TP
        self.noc_max = self.nrmax[0]

        self.dup_segs = []  # (r, base, width)
        wb = 0
        for r in range(1, RMAX0 + 1):
            w = self.nrmax[r - 1] - self.nrmin[r - 1]
            if w > 0:
                self.dup_segs.append((r, wb, w))
                wb += w
        self.W_SUB = ceil128(wb)

        TP = self.TP
        self.tok_of_pos = np.zeros((B, TP), np.int64)
        self.real = np.zeros((B, TP), bool)
        self.dup_ids = np.full((B, self.W_SUB), self.ZROW, np.int64)
        for b in range(B):
            pr = pr0_list[b]
            for r in range(1, RMAX0 + 1):
                nb = pr.nr(r)
                o = self.off[r - 1]
                w = self.nrmax[r - 1]
                if nb:
                    self.tok_of_pos[b, o:o + nb] = [
                        pr.members[s][r - 1] for s in range(nb)]
                    self.real[b, o:o + nb] = True
                for s in range(nb, w):
                    if r >= 2 or s < pr.n_occ:
                        self.tok_of_pos[b, o + s] = pr.members[s][0]
            for r, base, w in self.dup_segs:
                nb = pr.nr(r)
                lo = self.nrmin[r - 1]
                for j in range(w):
                    s = lo + j
                    if s >= nb:
                        self.dup_ids[b, base + j] = self.off[r - 1] + s


class StripStream:
    """Flat concatenated strip layout for planes 1 and 2 (one idx stream)."""
    def __init__(self, preps_q, zrow):
        # preps_q: {q: [prep per core]}
        self.zrow = zrow
        self.segs = []  # (q, r, width, stream_off)
        off = 0
        for q in (1, 2):
            prs = preps_q[q]
            rmax = max(pr.R_max for pr in prs)
            for r in range(1, rmax + 1):
                w = max(pr.nr(r) for pr in prs) if r > 1 else \
                    ceil128(max(pr.n_occ for pr in prs))
                self.segs.append((q, r, w, off))
                off += w
        self.width = ceil128(off)

    def ids(self, preps_q, b, sum_pad_all):
        out = np.full(self.width, self.zrow, np.int64)
        for q, r, w, off in self.segs:
            pr = preps_q[q][b]
            sp = True if r == 1 else sum_pad_all
            out[off:off + w] = pr.round_ids(r, w, sp, self.zrow)
        return out


def _build(inputs, prep, REPS=1, timing=False):
    import concourse.bacc as bacc
    import concourse.tile as tile
    from concourse import mybir

    sg: Sigma = prep["sigma"]
    prq = prep["prq"]        # {q: [Prep per core]}
    stream: StripStream = prep["stream"]
    TP, ZROW = sg.TP, sg.ZROW
    TPR = TP // 128
    NOC0 = ceil128(sg.noc_max)
    NOCQ = {q: ceil128(max(pr.n_occ for pr in prq[q])) for q in (1, 2)}
    TBR = {1: 0, 2: NOCQ[1] // 128}   # tbl2 rank base per plane
    NTBR = NOCQ[1] // 128 + NOCQ[2] // 128
    import os
    SW = stream.width
    CW = int(os.environ.get("K_CW", 1536))    # strip gather chunk width
    EXCH = int(os.environ.get("K_EXCH", 1024))  # expand chunk (positions)
    SRB = int(os.environ.get("K_SRB", 2))
    GPB = int(os.environ.get("K_GPB", 2))
    assert 2 * max(NOC0, NOCQ[1], NOCQ[2]) <= 2 * CW

    # ---- host-side weight prep (bias deferral), identical to before ----
    def stream_host(pref, base_bias):
        w0 = np.asarray(inputs[f"{pref}_w0"], F32)
        b0 = np.asarray(inputs[f"{pref}_b0"], F32)
        w1 = np.asarray(inputs[f"{pref}_w1"], F32)
        b1 = np.asarray(inputs[f"{pref}_b1"], F32)
        ws = np.asarray(inputs[f"{pref}_ws"], F32)
        relu_bias = []
        Bp = base_bias
        for i in range(NB):
            if i == 0:
                bias_in = Bp
                relu_bias.append((bias_in[:H].copy(), bias_in[H:].copy()))
            else:
                bias_in = np.concatenate([Bp, 3.0 * Bp])
                relu_bias.append((Bp.copy(), 3.0 * Bp))
            Bp = b1[i] + bias_in @ ws[i]
        return dict(w0=w0, b0=b0, w1=w1, ws=ws, relu_bias=relu_bias, B_final=Bp)

    wp = np.asarray(inputs["wp"], F32)
    bp = np.asarray(inputs["bp"], F32)
    wp2 = np.asarray(inputs["wp2"], F32)
    bp2 = np.asarray(inputs["bp2"], F32)
    sh_host = {"g": stream_host("blk", bp.copy()), "c": stream_host("blkc", bp + bp2)}
    fc_w = {"g": np.asarray(inputs["fc_c_w"], F32),
            "c": np.asarray(inputs["fc_cc_w"], F32)}
    fc_b = {"g": np.asarray(inputs["fc_c_b"], F32),
            "c": np.asarray(inputs["fc_cc_b"], F32)}
    cvec = {s: sh_host[s]["B_final"] @ fc_w[s] + fc_b[s] for s in ("g", "c")}

    nc = bacc.Bacc("TRN2", target_bir_lowering=False, debug=False, num_devices=B)
    dt = mybir.dt

    def din(name, shape, dtype):
        return nc.dram_tensor(name, shape, dtype, kind="ExternalInput")

    pT_d = din("pT", [3, TP], dt.bfloat16)
    p2T_d = din("p2T", [3, TP], dt.bfloat16)
    wp_d = din("wp", [3, 2 * H], dt.bfloat16)
    wp2_d = din("wp2", [3, 2 * H], dt.bfloat16)
    wpk_d = {}
    for s in ("g", "c"):
        wpk_d[s] = dict(
            w0=din(f"{s}_w0", [H, NB, 2 * H], dt.bfloat16),
            w1=din(f"{s}_w1", [H, NB, H], dt.bfloat16),
            ws=din(f"{s}_ws", [H, NB, 2 * H], dt.bfloat16),
            rb=din(f"{s}_rb", [H, NB, 2], dt.float32),
            b0=din(f"{s}_b0", [H, NB], dt.float32),
            fcw=din(f"{s}_fcw", [H, C], dt.bfloat16),
        )
    smax_d = din("smax", [128, SW // 16], dt.int16)
    smean_d = din("smean", [128, SW // 16], dt.int16)
    epidx_d = din("epidx", [128, 2 * TP // 16], dt.int16)
    dup_d = din("dup", [128, sg.W_SUB // 16], dt.int16)

    out_kind = "Internal" if timing else "ExternalOutput"
    sums_d = [nc.dram_tensor("sums_0", [128, NOC0 // 128, 2, C], dt.float32,
                             kind=out_kind)]
    for q in (1, 2):
        sums_d.append(nc.dram_tensor(f"sums_{q}", [128, NOCQ[q] // 128, 2, C],
                                     dt.float32, kind=out_kind))
    dups_d = nc.dram_tensor("dups", [128, sg.W_SUB // 128, 2, C], dt.float32,
                            kind=out_kind)
    chk_d = nc.dram_tensor("chk", [128, 128], dt.bfloat16, kind="ExternalOutput") \
        if timing else None

    with tile.TileContext(nc) as tc:
        with tc.tile_pool(name="const", bufs=1) as constp, \
             tc.tile_pool(name="act", bufs=1) as actp, \
             tc.tile_pool(name="pooledp", bufs=1) as pooledp, \
             tc.tile_pool(name="s12p", bufs=1) as s12p, \
             tc.tile_pool(name="small", bufs=2) as smallp, \
             tc.tile_pool(name="sr", bufs=SRB) as srp, \
             tc.tile_pool(name="gp", bufs=GPB) as gp, \
             tc.tile_pool(name="npm", bufs=1) as npmp, \
             tc.tile_pool(name="tblp", bufs=1) as tblp, \
             tc.tile_pool(name="meanp", bufs=1) as meanp, \
             tc.tile_pool(name="psum", bufs=2, space="PSUM") as psump:

            wp_t = constp.tile([3, 2 * H], dt.bfloat16)
            wp2_t = constp.tile([3, 2 * H], dt.bfloat16)
            nc.sync.dma_start(wp_t[:], wp_d[:])
            nc.sync.dma_start(wp2_t[:], wp2_d[:])
            W = {}
            for s in ("g", "c"):
                W[s] = dict(
                    w0=constp.tile([H, NB, 2 * H], dt.bfloat16, tag=f"{s}w0", name=f"{s}w0"),
                    w1=constp.tile([H, NB, H], dt.bfloat16, tag=f"{s}w1", name=f"{s}w1"),
                    ws=constp.tile([H, NB, 2 * H], dt.bfloat16, tag=f"{s}ws", name=f"{s}ws"),
                    rb=constp.tile([H, NB, 2], dt.float32, tag=f"{s}rb", name=f"{s}rb"),
                    b0=constp.tile([H, NB], dt.float32, tag=f"{s}b0", name=f"{s}b0"),
                    fcw=constp.tile([H, C], dt.bfloat16, tag=f"{s}fcw", name=f"{s}fcw"),
                )
                for k, t in W[s].items():
                    nc.sync.dma_start(t[:], wpk_d[s][k][:])
            smax_t = constp.tile([128, SW // 16], dt.int16, tag="smax", name="smax")
            smean_t = constp.tile([128, SW // 16], dt.int16, tag="smean", name="smean")
            epidx_t = constp.tile([128, 2 * TP // 16], dt.int16, tag="epidx", name="epidx")
            dup_t = constp.tile([128, sg.W_SUB // 16], dt.int16, tag="dup", name="dup")
            nc.sync.dma_start(smax_t[:], smax_d[:])
            nc.sync.dma_start(smean_t[:], smean_d[:])
            nc.sync.dma_start(epidx_t[:], epidx_d[:])
            nc.sync.dma_start(dup_t[:], dup_d[:])

            def sbuf_gather2(dst_ap, src_pm, idxs_ap, n):
                nc.gpsimd.dma_gather(
                    dst_ap, src_pm, idxs_ap, n, n, 2 * H,
                    transpose=True, single_packet=False,
                    sbuf_tokens_per_rank=128,
                    sbuf_free_dim_per_rank=2 * H * 2,
                )

            def sgview(flat_tile, w):
                return flat_tile[:, :2 * w].rearrange("p (s w) -> p s w", s=2)

            def resblock_chunk(s, i, xa, xb, sl):
                w = W[s]
                ba_ap = w["rb"][:, i, 0:1]
                bb_ap = w["rb"][:, i, 1:2]
                ra = smallp.tile([H, 512], dt.bfloat16, tag="ra", name="ra")
                rb_ = smallp.tile([H, 512], dt.bfloat16, tag="rb", name="rb")
                nc.vector.tensor_scalar(out=ra[:], in0=xa[:, sl], scalar1=ba_ap,
                                        scalar2=0.0, op0=mybir.AluOpType.add,
                                        op1=mybir.AluOpType.max)
                nc.vector.tensor_scalar(out=rb_[:], in0=xb[:, sl], scalar1=bb_ap,
                                        scalar2=0.0, op0=mybir.AluOpType.add,
                                        op1=mybir.AluOpType.max)
                ph = psump.tile([H, 512], dt.float32, tag="ph", name="ph")
                nc.tensor.matmul(ph[:], w["w0"][:, i, :H], ra[:],
                                 start=True, stop=False)
                nc.tensor.matmul(ph[:], w["w0"][:, i, H:], rb_[:],
                                 start=False, stop=True)
                h = smallp.tile([H, 512], dt.bfloat16, tag="h", name="h")
                nc.scalar.activation(h[:], ph[:], mybir.ActivationFunctionType.Relu,
                                     bias=w["b0"][:, i:i + 1], scale=1.0)
                po = psump.tile([H, 512], dt.float32, tag="po", name="po")
                nc.tensor.matmul(po[:], w["w1"][:, i, :], h[:],
                                 start=True, stop=False)
                nc.tensor.matmul(po[:], w["ws"][:, i, :H], xa[:, sl],
                                 start=False, stop=False)
                nc.tensor.matmul(po[:], w["ws"][:, i, H:], xb[:, sl],
                                 start=False, stop=True)
                nc.scalar.activation(xa[:, sl], po[:],
                                     mybir.ActivationFunctionType.Copy)

            # ---------------- schedule ----------------
            def one_rep():
                net2 = actp.tile([H, 2, TP], dt.bfloat16, tag="net2", name="net2")
                pooled2 = pooledp.tile([128, 2, TP], dt.bfloat16,
                                       tag="pooled2", name="pooled2")
                npm2 = npmp.tile([128, TPR + 1, 2, H], dt.bfloat16,
                                 tag="npm", name="npm")
                nc.vector.memset(npm2[:, TPR, :, :], 0.0)

                def transpose_range(t0, t1):
                    for si in range(2):
                        nc.sync.dma_start_transpose(
                            npm2[:, t0 // 128:t1 // 128, si, :],
                            net2[:, si, t0:t1])

                # x0 projections (biases deferred)
                for nt in range(TP // 512):
                    sl = slice(nt * 512, (nt + 1) * 512)
                    pT_t = smallp.tile([3, 512], dt.bfloat16, tag="pt", name="pT_t")
                    p2T_t = smallp.tile([3, 512], dt.bfloat16, tag="pt2", name="p2T_t")
                    nc.sync.dma_start(pT_t[:], pT_d[:, sl])
                    nc.sync.dma_start(p2T_t[:], p2T_d[:, sl])
                    for m in range(2):
                        ps_g = psump.tile([H, 512], dt.float32, tag="ph", name="ps_g")
                        ps_c = psump.tile([H, 512], dt.float32, tag="po", name="ps_c")
                        nc.tensor.matmul(ps_g[:], wp_t[:, m * H:(m + 1) * H],
                                         pT_t[:], start=True, stop=True)
                        nc.tensor.matmul(ps_c[:], wp2_t[:, m * H:(m + 1) * H],
                                         p2T_t[:], start=True, stop=True)
                        dst_g = net2[:, 0, sl] if m == 0 else pooled2[:, 0, sl]
                        dst_c = net2[:, 1, sl] if m == 0 else pooled2[:, 1, sl]
                        nc.scalar.activation(dst_g, ps_g[:],
                                             mybir.ActivationFunctionType.Copy)
                        nc.vector.tensor_tensor(out=dst_c, in0=dst_g,
                                                in1=ps_c[:], op=mybir.AluOpType.add)
                # block 0 + npm2 transposes
                for t0 in range(0, TP, EXCH):
                    t1 = min(t0 + EXCH, TP)
                    for nt in range(t0 // 512, t1 // 512):
                        sl = slice(nt * 512, (nt + 1) * 512)
                        for si, s in enumerate(("g", "c")):
                            resblock_chunk(s, 0, net2[:, si, :],
                                           pooled2[:, si, :], sl)
                    transpose_range(t0, t1)

                s12 = {}
                for i in range(1, NB):
                    # ---- plane 0: pure DVE ----
                    s12[0] = s12p.tile([128, 2, NOC0], dt.bfloat16,
                                       tag="s12_0", name="s12_0")
                    nc.vector.tensor_copy(s12[0][:, :, :sg.noc_max],
                                          net2[:, :, :sg.noc_max])
                    for r in range(2, sg.RMAX0 + 1):
                        w = sg.nrmax[r - 1]
                        o = int(sg.off[r - 1])
                        nc.vector.tensor_tensor(
                            out=s12[0][:, :, :w], in0=s12[0][:, :, :w],
                            in1=net2[:, :, o:o + w], op=mybir.AluOpType.max)
                    # ---- planes 1/2: strip gathers + seg maxes ----
                    for q in (1, 2):
                        s12[q] = s12p.tile([128, 2, NOCQ[q]], dt.bfloat16,
                                           tag=f"s12_{q}", name=f"s12_{q}")
                    for c0 in range(0, SW, CW):
                        wch = min(CW, SW - c0)
                        srf = srp.tile([128, 2 * CW], dt.bfloat16,
                                       tag="sr", name="sr")
                        sr2 = sgview(srf, wch)
                        sbuf_gather2(sr2, npm2[:],
                                     smax_t[:, c0 // 16:(c0 + wch) // 16], wch)
                        for qq, r, w, soff in stream.segs:
                            a = max(soff, c0)
                            bnd = min(soff + w, c0 + wch)
                            if a >= bnd:
                                continue
                            dst = s12[qq][:, :, a - soff:bnd - soff]
                            src = sr2[:, :, a - c0:bnd - c0]
                            if r == 1:
                                nc.vector.tensor_copy(dst, src)
                            else:
                                nc.vector.tensor_tensor(
                                    out=dst, in0=dst, in1=src,
                                    op=mybir.AluOpType.max)
                    # ---- tbl2 (stacked planes, PM) ----
                    tbl2 = tblp.tile([128, NTBR, 2, H], dt.bfloat16,
                                     tag="tbl2", name="tbl2")
                    for q in (1, 2):
                        for si in range(2):
                            nc.sync.dma_start_transpose(
                                tbl2[:, TBR[q]:TBR[q] + NOCQ[q] // 128, si, :],
                                s12[q][:, si, :])
                    # ---- plane 0 expand (slice copies) ----
                    for r in range(1, sg.RMAX0 + 1):
                        w = sg.nrmax[r - 1]
                        o = int(sg.off[r - 1])
                        nc.vector.tensor_copy(pooled2[:, :, o:o + w],
                                              s12[0][:, :, :w])
                    if TP > sg.STRUCT_END:
                        nc.vector.memset(pooled2[:, :, sg.STRUCT_END:TP], 0.0)
                    # ---- expand planes 1/2 + next resblock, chunk-pipelined ----
                    for t0 in range(0, TP, EXCH):
                        t1 = min(t0 + EXCH, TP)
                        e = t1 - t0
                        g2f = gp.tile([128, 4 * EXCH], dt.bfloat16,
                                      tag="g2", name="g2")
                        g2v = sgview(g2f, 2 * e)
                        sbuf_gather2(g2v, tbl2[:],
                                     epidx_t[:, 2 * t0 // 16:2 * t1 // 16],
                                     2 * e)
                        nc.vector.tensor_tensor(
                            out=pooled2[:, :, t0:t1],
                            in0=pooled2[:, :, t0:t1],
                            in1=g2v[:, :, :e], op=mybir.AluOpType.add)
                        nc.vector.tensor_tensor(
                            out=pooled2[:, :, t0:t1],
                            in0=pooled2[:, :, t0:t1],
                            in1=g2v[:, :, e:2 * e], op=mybir.AluOpType.add)
                        for nt in range(t0 // 512, t1 // 512):
                            sl = slice(nt * 512, (nt + 1) * 512)
                            for si, s in enumerate(("g", "c")):
                                resblock_chunk(s, i, net2[:, si, :],
                                               pooled2[:, si, :], sl)
                        transpose_range(t0, t1)

                # ---------------- mean stage ----------------
                ACCW = max(NOC0, NOCQ[1], NOCQ[2])
                accf = meanp.tile([128, 2 * ACCW], dt.float32,
                                  tag="acc", name="acc")

                def emit_sums(acc_ap, width, dram, dram_base):
                    accb_f = srp.tile([128, 2 * CW], dt.bfloat16,
                                      tag="sr", name="accb")
                    accb = sgview(accb_f, width)
                    nc.vector.tensor_copy(accb[:], acc_ap)
                    nch_total = width // 128
                    for ch2 in range((nch_total + 1) // 2):
                        nch = min(2, nch_total - ch2 * 2)
                        sums = meanp.tile([128, 2, 2, C], dt.float32,
                                          tag="sums", name="sums")
                        for si, s in enumerate(("g", "c")):
                            pb = psump.tile([128, 512], dt.float32,
                                            tag="ph", name="pb")
                            for k in range(nch):
                                ch = ch2 * 2 + k
                                nc.tensor.matmul(
                                    pb[:, k * C:(k + 1) * C],
                                    accb[:, si, ch * 128:(ch + 1) * 128],
                                    W[s]["fcw"][:], start=True, stop=True)
                            nc.vector.tensor_copy(
                                sums[:, :nch, si, :],
                                pb[:, :nch * C].rearrange(
                                    "p (a f) -> p a f", a=nch))
                        nc.sync.dma_start(
                            dram[:, dram_base + ch2 * 2:dram_base + ch2 * 2 + nch,
                                 :, :],
                            sums[:, :nch, :, :])

                # plane 0: slice prefix sums in fp32
                acc0 = sgview(accf, NOC0)
                nc.vector.tensor_copy(acc0[:, :, :sg.noc_max],
                                      net2[:, :, :sg.noc_max])
                if NOC0 > sg.noc_max:
                    nc.vector.memset(acc0[:, :, sg.noc_max:], 0.0)
                for r in range(2, sg.RMAX0 + 1):
                    w = sg.nrmax[r - 1]
                    o = int(sg.off[r - 1])
                    nc.vector.tensor_tensor(
                        out=acc0[:, :, :w], in0=acc0[:, :, :w],
                        in1=net2[:, :, o:o + w], op=mybir.AluOpType.add)
                emit_sums(acc0[:], NOC0, sums_d[0], 0)

                # dup correction strip
                dupg = meanp.tile([128, 2, sg.W_SUB], dt.bfloat16,
                                  tag="dupg", name="dupg")
                sbuf_gather2(dupg[:], npm2[:], dup_t[:], sg.W_SUB)
                emit_sums(dupg[:], sg.W_SUB, dups_d, 0)

                # planes 1/2: gathered prefix sums (one pass per plane; a pass
                # only gathers chunks that intersect that plane's segments)
                for q in (1, 2):
                    accq = sgview(accf, NOCQ[q])
                    for c0 in range(0, SW, CW):
                        wch = min(CW, SW - c0)
                        need = any(
                            qq == q and max(soff, c0) < min(soff + w, c0 + wch)
                            for qq, r, w, soff in stream.segs)
                        if not need:
                            continue
                        srf = srp.tile([128, 2 * CW], dt.bfloat16,
                                       tag="sr", name="srm")
                        sr2 = sgview(srf, wch)
                        sbuf_gather2(sr2, npm2[:],
                                     smean_t[:, c0 // 16:(c0 + wch) // 16], wch)
                        for qq, r, w, soff in stream.segs:
                            if qq != q:
                                continue
                            a = max(soff, c0)
                            bnd = min(soff + w, c0 + wch)
                            if a >= bnd:
                                continue
                            dst = accq[:, :, a - soff:bnd - soff]
                            src = sr2[:, :, a - c0:bnd - c0]
                            if r == 1:
                                nc.vector.tensor_copy(dst, src)
                            else:
                                nc.vector.tensor_tensor(
                                    out=dst, in0=dst, in1=src,
                                    op=mybir.AluOpType.add)
                    emit_sums(accq[:], NOCQ[q], sums_d[q], 0)

                return net2

            if timing:
                with tc.For_i(0, REPS):
                    net2 = one_rep()
            else:
                net2 = one_rep()

            if timing:
                chk_t = constp.tile([128, 128], dt.bfloat16)
                nc.vector.tensor_copy(chk_t[:], net2[:, 0, :128])
                nc.sync.dma_start(chk_d[:], chk_t[:])

    nc.compile()

    # ---------------- input maps ----------------
    p = np.asarray(inputs["p"], F32)
    p2 = np.asarray(inputs["p2"], F32)
    in_maps = []
    for b in range(B):
        tp = p[b][sg.tok_of_pos[b]]
        tp2 = p2[b][sg.tok_of_pos[b]]
        im = {
            "pT": np.ascontiguousarray(tp.T).astype(BF),
            "p2T": np.ascontiguousarray(tp2.T).astype(BF),
            "wp": wp.astype(BF), "wp2": wp2.astype(BF),
        }
        for s in ("g", "c"):
            sh = sh_host[s]
            w0pk = np.concatenate([sh["w0"][:, :H].transpose(1, 0, 2),
                                   sh["w0"][:, H:].transpose(1, 0, 2)], axis=2)
            wspk = np.concatenate([sh["ws"][:, :H].transpose(1, 0, 2),
                                   sh["ws"][:, H:].transpose(1, 0, 2)], axis=2)
            w1pk = sh["w1"].transpose(1, 0, 2)
            rb = np.zeros((H, NB, 2), F32)
            for i, (ba, bb) in enumerate(sh["relu_bias"]):
                rb[:, i, 0] = ba
                rb[:, i, 1] = bb
            im[f"{s}_w0"] = np.ascontiguousarray(w0pk).astype(BF)
            im[f"{s}_w1"] = np.ascontiguousarray(w1pk).astype(BF)
            im[f"{s}_ws"] = np.ascontiguousarray(wspk).astype(BF)
            im[f"{s}_rb"] = rb
            im[f"{s}_b0"] = np.ascontiguousarray(sh["b0"].T).astype(F32)
            im[f"{s}_fcw"] = fc_w[s].astype(BF)
        im["smax"] = wrap_idxs(stream.ids(prq, b, sum_pad_all=False))
        im["smean"] = wrap_idxs(stream.ids(prq, b, sum_pad_all=True))
        ep = np.empty(2 * TP, np.int64)
        for t0 in range(0, TP, EXCH):
            t1 = min(t0 + EXCH, TP)
            e = t1 - t0
            ep[2 * t0:2 * t0 + e] = prq[1][b].pidx[t0:t1] + TBR[1] * 128
            ep[2 * t0 + e:2 * t1] = prq[2][b].pidx[t0:t1] + TBR[2] * 128
        im["epidx"] = wrap_idxs(ep)
        im["dup"] = wrap_idxs(sg.dup_ids[b])
        in_maps.append(im)

    return nc, in_maps, cvec


def _prep(inputs):
    p = np.asarray(inputs["p"], F32)
    pr0 = [Prep(compute_idx_lists(p[b])[0], np.ones(T, bool)) for b in range(B)]
    sg = Sigma(pr0)
    prq = {1: [], 2: []}
    for b in range(B):
        idx_lists = compute_idx_lists(p[b][sg.tok_of_pos[b]])
        for q in (1, 2):
            prq[q].append(Prep(idx_lists[q], sg.real[b]))
    stream = StripStream(prq, sg.ZROW)
    return {"sigma": sg, "prq": prq, "stream": stream}


def kernel(**inputs):
    from concourse.bass_utils import run_bass_kernel_spmd

    prep = _prep(inputs)
    sg: Sigma = prep["sigma"]
    prq = prep["prq"]
    nc, in_maps, cvec = _build(inputs, prep, REPS=1, timing=False)
    res = run_bass_kernel_spmd(nc, in_maps, core_ids=list(range(B)))

    out = np.zeros((2 * len(PLANE_COLS), B, C, R, R), F32)
    for b in range(B):
        dups = np.asarray(res.results[b]["dups"], F32)
        dupranks = dups.transpose(1, 0, 2, 3).reshape(-1, 2, C)
        for pl in range(3):
            pr = sg.pr0[b] if pl == 0 else prq[pl][b]
            compact = np.asarray(res.results[b][f"sums_{pl}"], F32)
            ranks = compact.transpose(1, 0, 2, 3).reshape(-1, 2, C).copy()
            if pl == 0:
                for r, base, w in sg.dup_segs:
                    lo = sg.nrmin[r - 1]
                    ranks[lo:lo + w] -= dupranks[base:base + w]
            cnt = pr.cnt.astype(F32)
            for si, s in enumerate(("g", "c")):
                grid = np.zeros((R * R, C), F32)
                grid[pr.bins_sorted] = ranks[:pr.n_occ, si]
                true_sums = grid + cnt[:, None] * cvec[s][None, :]
                mean = true_sums / np.clip(cnt, 1.0, None)[:, None]
                mean[cnt == 0] = 0.0
                out[si * 3 + pl, b] = mean.T.reshape(C, R, R)
    return out


def measure_hw_time(inputs, reps=1000, n_timing_runs=8):
    import time
    from concourse.bass_utils import run_bass_kernel_spmd

    prep = _prep(inputs)

    def runner(R_):
        nc, in_maps, _ = _build(inputs, prep, REPS=R_, timing=True)

        def once():
            t0 = time.perf_counter()
            run_bass_kernel_spmd(nc, in_maps, core_ids=list(range(B)))
            return time.perf_counter() - t0
        once()
        return min(once() for _ in range(n_timing_runs))

    t1 = runner(1)
    tR = runner(reps)
    per_iter = (tR - t1) / (reps - 1)
    return int(per_iter * 1e9), t1, tR


if __name__ == "__main__":
    import reference
    inputs = {k: np.asarray(v) for k, v in reference.setup_inputs().items()}
    result = kernel(**inputs)
    print("kernel output shape:", result.shape)
